# revision 17
# baseline (speedup 1.0000x reference)
"""Trainium2 Bass kernel for nn_CCepLTVFilter (v2).

Pipeline (per core, frequency-sharded f-slice of 128 across 8 cores):
  1. conv1d(x, W) + b     -> ccep[o, bt]        PE: 3 taps x 2 o-chunks, fp16
  2. Yr/Yi = DFT(ccep)    -> [f, bt]            PE, lhsT = CF/SF fp16
  3. mag = exp(Yr) (ln10/10 folded into CF); sin/cos(Yi) via ACT Sin
  4. Zr/Zi = 1025-pt DFT of z hops              PE: hop-matrix rhs, h-shifted
  5. P = (mag e^{i ph}) * (Zr + i Zi)           DVE complex mult (V+G split)
  6. ob[t, l|r] = P_b.T @ (CO|SO)               PE, fp16 moving N=512
  7. overlap-add of l/r output planes on HOST during gather (linear op)

All matmuls are uniform fp16 (1 cycle/row on PE); fp16 quantization of the
DFT matrices keeps rel err ~5e-3 (validated vs fp32 reference in sim).
z enters transposed from host as a hop matrix -> no PE transposes at all.
"""

import numpy as np
import ml_dtypes

import concourse.bass as bass
import concourse.bacc as bacc
import concourse.mybir as mybir
import concourse.tile as tile
from concourse.bass_utils import run_bass_kernel_spmd

# ---------------- problem dims (hardcoded) ----------------
B, T, D = 2, 128, 80
CCEP = 222
FFT = 1024
HOP = 256
WIN = 2 * HOP            # 512
PAD = (FFT - CCEP) // 2  # 401
M = FFT + 1              # 1025-point transforms
BT = B * T               # 256
NCORES = 8
FS = FFT // NCORES       # 128 frequencies per core
OC = CCEP // 2           # 111 (o-chunk)
LAM = float(np.log(10.0) / 10.0)
NWARM = 3                # PE warm-up matmuls (HAM clock ramp)

F32 = mybir.dt.float32
F16 = mybir.dt.float16
PI = float(np.pi)
AF = mybir.ActivationFunctionType
OP = mybir.AluOpType

TRACE = False            # set by test harness for profiling
LAST_RESULT = None       # BassKernelResults of last run (for test harness)


# ---------------- host-side constants (input independent) ----------------
def _make_constants():
    o = np.arange(CCEP, dtype=np.float64)[:, None]
    f = np.arange(FFT, dtype=np.float64)[None, :]
    qn_idx = np.arange(1, CCEP // 2 + 1, dtype=np.float64)
    qnorm = np.concatenate([qn_idx[::-1], qn_idx])
    ang = 2.0 * np.pi * f * (o + PAD) / FFT
    CF = (np.cos(ang) * LAM / qnorm[:, None]).astype(np.float16)   # [222,1024]
    SF = (-np.sin(ang) / qnorm[:, None]).astype(np.float16)

    u = np.arange(WIN, dtype=np.float64)[:, None]
    phi = 2.0 * np.pi * f * (u + FFT // 2) / M
    ZC = np.cos(phi).astype(np.float16)                            # [512,1024]
    ZS = np.sin(phi).astype(np.float16)

    w = np.arange(WIN, dtype=np.float64)[None, :]
    th = 2.0 * np.pi * np.arange(FFT, dtype=np.float64)[:, None] * w / M
    win = 0.5 * (1.0 - np.cos(2.0 * np.pi * np.arange(WIN) / WIN))
    CO = (np.cos(th) * win[None, :] / M).astype(np.float16)        # [1024,512]
    SO = (np.sin(th) * win[None, :] / M).astype(np.float16)

    consts = []
    for c in range(NCORES):
        sl = slice(c * FS, (c + 1) * FS)
        dpb = np.concatenate(
            [CF[0:OC, sl], CF[OC:CCEP, sl], SF[0:OC, sl], SF[OC:CCEP, sl]],
            axis=1).astype(np.float16)                             # [111, 512]
        zchunks = [ZC[h * 256 + vc * 128: h * 256 + (vc + 1) * 128, sl]
                   for h in range(2) for vc in range(2)]
        schunks = [ZS[h * 256 + vc * 128: h * 256 + (vc + 1) * 128, sl]
                   for h in range(2) for vc in range(2)]
        dpc = np.concatenate(zchunks + schunks, axis=1).astype(np.float16)
        dpd = np.concatenate([CO[sl, :], SO[sl, :]], axis=1).astype(np.float16)
        consts.append(dict(dpb=dpb, dpc=dpc, dpd=dpd))
    return consts


_CONSTS = _make_constants()
_NC = None


# ---------------- device program ----------------
def _build_nc():
    nc = bacc.Bacc()
    d1 = nc.dram_tensor("dpa1", [81, 1434], F16, kind="ExternalInput")
    d2 = nc.dram_tensor("dpa2", [128, 1024], F16, kind="ExternalInput")
    d3 = nc.dram_tensor("dpb", [OC, 512], F16, kind="ExternalInput")
    d4 = nc.dram_tensor("dpc", [128, 1024], F16, kind="ExternalInput")
    d5 = nc.dram_tensor("dpd", [128, 1024], F16, kind="ExternalInput")
    out_e = nc.dram_tensor("out", [B, 2, T * HOP], F16, kind="ExternalOutput")

    with tile.TileContext(nc) as tc:
        with tc.tile_pool(name="sb", bufs=1) as sb, \
             tc.tile_pool(name="ps", bufs=1, space="PSUM") as ps:

            # ---- input DMAs, ordered by first use ----
            dpa1 = sb.tile([81, 1434], F16, tag="dpa1", name="dpa1")
            nc.sync.dma_start(out=dpa1[:], in_=d1[:, :])
            dpa2 = sb.tile([128, 1024], F16, tag="dpa2", name="dpa2")
            nc.sync.dma_start(out=dpa2[:], in_=d2[:, :])
            dpb = sb.tile([OC, 512], F16, tag="dpb", name="dpb")
            nc.sync.dma_start(out=dpb[:], in_=d3[:, :])
            dpc = sb.tile([128, 1024], F16, tag="dpc", name="dpc")
            nc.gpsimd.dma_start(out=dpc[:], in_=d4[:, :])
            dpd = sb.tile([128, 1024], F16, tag="dpd", name="dpd")
            nc.gpsimd.dma_start(out=dpd[:], in_=d5[:, :])

            xp = dpa1[0:81, 0:768]                 # [81, 3*256] pre-shifted x
            wk = dpa1[0:81, 768:1434]              # [81, 3*222] conv taps

            # ---- PE warm-up (HAM ramp) + ACT table pre-load ----
            wsc = sb.tile([128, 256], F16, tag="wsc", name="wsc")
            nc.vector.memset(wsc[:, :], 0.0)
            tsc = sb.tile([1, 1], F32, tag="tsc", name="tsc")
            nc.scalar.activation(tsc[:, :], wsc[0:1, 0:1], AF.Sin)
            wps = ps.tile([128, 256], F32, tag="wps", name="wps")
            for i in range(NWARM):
                nc.tensor.matmul(wps[:, :], wsc[:, 0:128], wsc[:, :],
                                 start=True, stop=True)

            # ---- conv: ccep[o, bt] = sum_k Wk.T @ x(t+k-1), bias on k=1 ----
            convp = ps.tile([OC, 2 * BT], F32, tag="convp", name="convp")
            ccep_ps = []
            for oc in range(2):
                pc = convp[:, oc * BT:(oc + 1) * BT]
                for k in range(3):
                    lhs = wk[:, k * CCEP + oc * OC: k * CCEP + (oc + 1) * OC]
                    nc.tensor.matmul(pc, lhs, xp[:, k * BT:(k + 1) * BT],
                                     start=(k == 0), stop=(k == 2))
                ccep_ps.append(pc)
            ccep_sb = sb.tile([OC, 2 * BT], F16, tag="ccep_sb", name="ccep_sb")
            nc.vector.tensor_copy(ccep_sb[:, :], convp[:, :])
            ccep = [ccep_sb[:, 0:BT], ccep_sb[:, BT:2 * BT]]

            # ---- Yr/Yi [f_local, bt] ----
            yri = ps.tile([FS, 2 * BT], F32, tag="yri", name="yri")
            yr = yri[:, 0:BT]
            yi = yri[:, BT:2 * BT]
            for oc in range(2):
                nc.tensor.matmul(yr, dpb[:, oc * 128:(oc + 1) * 128],
                                 ccep[oc], start=(oc == 0), stop=(oc == 1))
            for oc in range(2):
                nc.tensor.matmul(yi, dpb[:, 256 + oc * 128:256 + (oc + 1) * 128],
                                 ccep[oc], start=(oc == 0), stop=(oc == 1))

            # ---- mag = exp(Yr); cos/sin(Yi) with range wrap ----
            mag = sb.tile([FS, BT], F32, tag="mag", name="mag")
            nc.scalar.activation(mag[:, :], yr, AF.Exp)
            yw1 = sb.tile([FS, BT], F32, tag="yw1", name="yw1")
            nc.vector.add_range_wrap(yw1[:, :], yi, PI / 2.0, PI, 2.0 * PI)
            yw2 = sb.tile([FS, BT], F32, tag="yw2", name="yw2")
            nc.vector.add_range_wrap(yw2[:, :], yi, 0.0, PI, 2.0 * PI)
            cosvt = sb.tile([FS, BT], F32, tag="cosvt", name="cosvt")
            nc.scalar.activation(cosvt[:, :], yw1[:, :], AF.Sin)
            sinvt = sb.tile([FS, BT], F32, tag="sinvt", name="sinvt")
            nc.scalar.activation(sinvt[:, :], yw2[:, :], AF.Sin)
            cosv = cosvt[:, :]
            sinv = sinvt[:, :]

            # ---- Zr/Zi [f_local, bt]: hop-DFT with h-shifted rhs ----
            zri = ps.tile([FS, 2 * BT], F32, tag="zri", name="zri")
            zr = zri[:, 0:BT]
            zi = zri[:, BT:2 * BT]
            chunks = [(h, vc) for h in range(2) for vc in range(2)]
            for i, (h, vc) in enumerate(chunks):
                rhs = dpa2[:, (2 * h + vc) * BT:(2 * h + vc + 1) * BT]
                nc.tensor.matmul(zr, dpc[:, (2 * h + vc) * 128:(2 * h + vc + 1) * 128],
                                 rhs, start=(i == 0), stop=(i == 3))
            for i, (h, vc) in enumerate(chunks):
                rhs = dpa2[:, (2 * h + vc) * BT:(2 * h + vc + 1) * BT]
                nc.tensor.matmul(zi, dpc[:, 512 + (2 * h + vc) * 128:512 + (2 * h + vc + 1) * 128],
                                 rhs, start=(i == 0), stop=(i == 3))

            # ---- P = mag e^{i ph} * (Zr + i Zi) ----
            # GPSIMD cannot touch PSUM, so V reads zr/zi; G gets SBUF-only ops.
            mzr = sb.tile([FS, BT], F32, tag="mzr", name="mzr")
            nc.vector.tensor_tensor(mzr[:, :], mag[:, :], zr, OP.mult)
            mzi = sb.tile([FS, BT], F32, tag="mzi", name="mzi")
            nc.vector.tensor_tensor(mzi[:, :], mag[:, :], zi, OP.mult)
            u1 = sb.tile([FS, BT], F32, tag="u1", name="u1")
            nc.vector.tensor_tensor(u1[:, :], cosv, mzr[:, :], OP.mult)
            u2 = sb.tile([FS, BT], F32, tag="u2", name="u2")
            nc.gpsimd.tensor_tensor(u2[:, :], sinv, mzi[:, :], OP.mult)
            t4 = sb.tile([FS, BT], F32, tag="t4", name="t4")
            nc.vector.tensor_tensor(t4[:, :], sinv, mzr[:, :], OP.mult)
            t3 = sb.tile([FS, BT], F32, tag="t3", name="t3")
            nc.gpsimd.tensor_tensor(t3[:, :], cosv, mzi[:, :], OP.mult)
            pr = sb.tile([FS, BT], F16, tag="pr", name="pr")
            nc.vector.tensor_tensor(pr[:, :], u1[:, :], u2[:, :], OP.subtract)
            pi = sb.tile([FS, BT], F16, tag="pi", name="pi")
            nc.gpsimd.tensor_tensor(pi[:, :], t3[:, :], t4[:, :], OP.add)

            # ---- ob[t, l|r] = P_b.T @ (CO|SO); OLA of planes happens on host ----
            for bb in range(B):
                obp = ps.tile([T, WIN], F32, tag=f"ob{bb}", name=f"ob{bb}")
                nc.tensor.matmul(obp[:, :], pr[:, bb * T:(bb + 1) * T],
                                 dpd[:, 0:512], start=True, stop=False)
                nc.tensor.matmul(obp[:, :], pi[:, bb * T:(bb + 1) * T],
                                 dpd[:, 512:1024], start=False, stop=True)
                obs = sb.tile([T, WIN], F16, tag=f"obs{bb}", name=f"obs{bb}")
                if bb == 0:
                    nc.scalar.copy(obs[:, :], obp[:, :])
                else:
                    nc.vector.tensor_copy(obs[:, :], obp[:, :])
                # dst[bb, plane, t*HOP + s] <- obs[t, plane*HOP + s]
                dst = bass.AP(out_e[:, :, :].tensor, bb * 2 * T * HOP,
                              [[HOP, T], [T * HOP, 2], [1, HOP]])
                eng = nc.sync if bb == 0 else nc.scalar
                eng.dma_start(out=dst, in_=obs[:, :])

    return nc


def _get_nc():
    global _NC
    if _NC is None:
        _NC = _build_nc()
        _NC.finalize()
    return _NC


# ---------------- host orchestration ----------------
def kernel(x, z, W, b):
    global LAST_RESULT
    x = np.asarray(x, dtype=np.float32)
    z = np.asarray(z, dtype=np.float32)
    W = np.asarray(W, dtype=np.float32)
    b = np.asarray(b, dtype=np.float32)

    # dpa1 = pre-shifted x copies [81, 3*256] | Wk [81, 3*222]
    xv = x.astype(np.float16)                                     # [2,128,80]
    dpa1 = np.zeros((81, 1434), np.float16)
    for k in range(3):
        blk = np.zeros((81, B, T), np.float16)
        lo, hi = max(0, 1 - k), min(T, T + 1 - k)                 # t+k-1 in range
        blk[0:80, :, lo:hi] = xv[:, lo + k - 1: hi + k - 1, :].transpose(2, 0, 1)
        if k == 1:
            blk[80] = 1.0                                         # bias row
        dpa1[:, k * BT:(k + 1) * BT] = blk.reshape(81, BT)
        dpa1[0:80, 768 + k * CCEP: 768 + (k + 1) * CCEP] = \
            W[:, :, k].T.astype(np.float16)
    dpa1[80, 768 + CCEP: 768 + 2 * CCEP] = b.astype(np.float16)   # bias, k=1

    # dpa2 = hop matrix, duplicated per h-shift: chunk (h,vc) at (2h+vc)*256
    zpad = np.concatenate(
        [np.zeros((B, HOP), np.float32), z[:, 0, :]], axis=1)     # [2, 33024]
    Hm = zpad.reshape(B, 129, HOP).transpose(2, 0, 1)             # [256, 2, 129]
    dpa2 = np.zeros((128, 1024), np.float16)
    for h in range(2):
        for vc in range(2):
            dpa2[:, (2 * h + vc) * BT:(2 * h + vc + 1) * BT] = \
                Hm[vc * 128:(vc + 1) * 128, :, h:h + 128].reshape(128, BT)

    shared = {"dpa1": dpa1, "dpa2": dpa2}
    in_maps = [{**shared, **_CONSTS[c]} for c in range(NCORES)]

    nc = _get_nc()
    res = run_bass_kernel_spmd(nc, in_maps, list(range(NCORES)), trace=TRACE)
    LAST_RESULT = res
    acc = np.zeros((B, 2, T * HOP), dtype=np.float32)
    for r in res.results:
        acc += np.asarray(r["out"], dtype=np.float32)
    out = np.empty((B, 1, T * HOP), dtype=np.float32)
    for bb in range(B):
        out[bb, 0] = acc[bb, 0] + np.roll(acc[bb, 1], HOP)
    return out
